# revision 47
# baseline (speedup 1.0000x reference)
# Multi-head attention (B=2, L=2048, D=1024, H=16, Dq=Dv=64) on 8 TRN2 NeuronCores.
#
# Sharding: data-parallel over (batch, query-rows). Core c owns batch c//4 and
# query window [(c%4)*512, (c%4)*512+512). Each core computes K/V projections
# for its batch (duplicated across the 4 cores of a batch group), its query
# projection, masked softmax attention and the output projection for its 512
# rows. No collectives; outputs are disjoint row blocks concatenated on host.
#
# v2 layout: all matmul operands in bf16 (fp32 PSUM accumulation), everything
# feature-major so the PE contracts along partitions with no on-chip
# transposes:
#   qT [d, q], kT [d, k]  (d on partitions, bf16)
#   v  [k, dv+1] bf16     (ones column makes AV row 64 accumulate Z for free)
#   S^T [k, q] = kT.T-contract-d qT  (two heads packed via PE row-tiling)
#   e = exp(S^T) bf16 (ACT, PSUM->SBUF); mask applied post-exp with one
#     copy_predicated (masked -> 1.0 == exp(1e-9), faithful to the reference)
#   AV^T [dv+1, q] accumulated over k-chunks in PSUM
#   1/Z via reciprocal_approx_fast (DVE custom op, ~18 bits), PE-broadcast
#     down to rows 0..63 via a contraction-1 f32r matmul, one tensor_mul
#   out [q, dm] = attnT.T-contract-hd Wo
# PSUM budget: st 2x[128,1024] (4 banks) + av0/av1 (2) + zbb (1) + 1 spare.
import numpy as np

B, L, DM, H, DQ = 2, 2048, 1024, 16, 64
P = 128
NC = 8
QW = (B * L) // NC          # 512 query rows per core
CC = DM // P                # 8 contraction chunks
HP = H // 2                 # 8 head pairs
KC = L // P                 # 16 key chunks

_CACHE = {}


def _build():
    import concourse.tile as tile
    from concourse import bacc, mybir

    f32 = mybir.dt.float32
    f32r = mybir.dt.float32r
    bf16 = mybir.dt.bfloat16
    u8 = mybir.dt.uint8
    fp8 = mybir.dt.float8e4
    Exp = mybir.ActivationFunctionType.Exp
    DR = mybir.MatmulPerfMode.DoubleRow

    nc = bacc.Bacc("TRN2", target_bir_lowering=False, debug=False,
                   enable_asserts=False, num_devices=NC)

    qt = nc.dram_tensor("qt", [DM, QW], bf16, kind="ExternalInput").ap()
    kt = nc.dram_tensor("kt", [DM, L], bf16, kind="ExternalInput").ap()
    vt = nc.dram_tensor("vt", [DM, L], bf16, kind="ExternalInput").ap()
    wq = nc.dram_tensor("wq", [DM, DM], bf16, kind="ExternalInput").ap()
    wk = nc.dram_tensor("wk", [DM, DM], bf16, kind="ExternalInput").ap()
    wv = nc.dram_tensor("wv", [DM, DM], bf16, kind="ExternalInput").ap()
    wo = nc.dram_tensor("wo", [DM, DM], bf16, kind="ExternalInput").ap()
    mkt = nc.dram_tensor("mkt", [H, L, QW], u8, kind="ExternalInput").ap()
    out = nc.dram_tensor("out", [QW, DM], f32, kind="ExternalOutput").ap()

    qt_r = qt.rearrange("(cc p) q -> p cc q", p=P)
    kt_r = kt.rearrange("(cc p) k -> p cc k", p=P)
    vt_r = vt.rearrange("(cc p) k -> p cc k", p=P)
    wq_r = wq.rearrange("(cc p) d -> p cc d", p=P)
    wk_r = wk.rearrange("(cc p) d -> p cc d", p=P)
    wv_r = wv.rearrange("(cc p) d -> p cc d", p=P)
    wo_r = wo.rearrange("(cc p) d -> p cc d", p=P)
    # mask grouped as [p, ko, h, kk, q]: one 3-dim DMA per head fetches 4
    # k-chunks at once (256 KB)
    mkt_r = mkt.rearrange("h (ko kk p) q -> p ko h kk q", kk=4, p=P)
    # GPSIMD mask offload abandoned: generic TensorTensor/TensorScalar BIR
    # ops fail the Pool-engine ISA check in this toolchain.
    OFFKC = ()

    with tile.TileContext(nc) as tc:
        from contextlib import ExitStack
        with ExitStack() as top:
            persist = top.enter_context(tc.tile_pool(name="persist", bufs=1))
            # v projection with an extra all-ones column: AV matmul row 64
            # then accumulates Z = sum_k e[k, q] for free.
            vproj = persist.tile([P, KC, H, DQ + 1], bf16)   # 32.5 KB/part
            kproj = persist.tile([P, CC, L], bf16)           # 32 KB/part
            qproj = persist.tile([P, CC, QW], bf16)          # 8 KB/part
            attnT = persist.tile([P, HP, QW], bf16)          # 8 KB/part
            wo_sb = persist.tile([P, CC, DM], bf16)          # 16 KB/part
            ones = persist.tile([P, DQ], f32)
            c1f = persist.tile([P, 1], f32)
            c1b = persist.tile([P, 1], bf16)
            nc.vector.memset(ones[:], 1.0)
            nc.vector.memset(c1f[:], 1.0)
            nc.vector.memset(c1b[:], 1.0)
            nc.vector.tensor_copy(
                vproj[:, :, :, DQ:DQ + 1],
                c1f[:, 0:1].to_broadcast([P, KC, H, 1]))
            for cc in range(CC):
                nc.sync.dma_start(wo_sb[:, cc, :], wo_r[:, cc, :])

            # scores PSUM pool hoisted above the projection pool so it gets
            # disjoint banks: the first head-pair's scores/exp/mask pipeline
            # can then run concurrently with the tail of phase C.
            stp = top.enter_context(
                tc.tile_pool(name="st", bufs=2, space="PSUM"))

            # ---- phases A/B/C: projections, weight-stationary, long streams
            with ExitStack() as ctx:
                wpool = ctx.enter_context(tc.tile_pool(name="wstage", bufs=2))
                spool = ctx.enter_context(tc.tile_pool(name="astage", bufs=2))
                ppool = ctx.enter_context(
                    tc.tile_pool(name="pproj", bufs=2, space="PSUM"))

                # phase A: qproj[d, q] = WQ.T-contract-c Q^T
                wqt = wpool.tile([P, CC, DM], bf16, tag="w", name="wq")
                for cc in range(CC):
                    nc.sync.dma_start(wqt[:, cc, :], wq_r[:, cc, :])
                a0 = spool.tile([P, CC, 2 * QW], bf16, tag="act", name="aq")
                for cc in range(CC):
                    nc.sync.dma_start(a0[:, cc, 0:QW], qt_r[:, cc, :])
                for dp in range(CC // 2):
                    ps = ppool.tile([P, 1024], f32, tag="ps")
                    for half in range(2):
                        dc = 2 * dp + half
                        sl = slice(half * QW, (half + 1) * QW)
                        for cc in range(CC):
                            nc.tensor.matmul(ps[:, sl],
                                             wqt[:, cc, dc * P:(dc + 1) * P],
                                             a0[:, cc, 0:QW],
                                             start=(cc == 0), stop=(cc == CC - 1))
                    nc.scalar.copy(
                        qproj[:, 2 * dp:2 * dp + 2, :],
                        ps[:].rearrange("p (c q) -> p c q", c=2))

                # phase B: kproj[d, k], k-blocks of 1024 (bf16 streams)
                wkt = wpool.tile([P, CC, DM], bf16, tag="w", name="wk")
                for cc in range(CC):
                    nc.sync.dma_start(wkt[:, cc, :], wk_r[:, cc, :])
                KB = 1024
                for kb in range(L // KB):
                    ksl = slice(kb * KB, (kb + 1) * KB)
                    a_sb = spool.tile([P, CC, KB], bf16, tag="act")
                    nc.sync.dma_start(a_sb[:], kt_r[:, :, ksl])
                    for dc in range(CC):
                        ps = ppool.tile([P, KB], f32, tag="ps")
                        for half in range(2):
                            hs = slice(half * 512, (half + 1) * 512)
                            for cc in range(CC):
                                nc.tensor.matmul(ps[:, hs],
                                                 wkt[:, cc, dc * P:(dc + 1) * P],
                                                 a_sb[:, cc, hs],
                                                 start=(cc == 0), stop=(cc == CC - 1))
                        nc.scalar.copy(kproj[:, dc, ksl], ps[:])

                # phase C: v[k, dv] = V^T as lhsT, WV as rhs
                wvt = wpool.tile([P, CC, DM], bf16, tag="w", name="wv")
                for cc in range(CC):
                    nc.sync.dma_start(wvt[:, cc, :], wv_r[:, cc, :])
                for kb in range(L // KB):
                    a_sb = spool.tile([P, CC, KB], bf16, tag="act")
                    nc.sync.dma_start(a_sb[:], vt_r[:, :, kb * KB:(kb + 1) * KB])
                    for kq in range(KB // P):
                        kc = (kb * KB) // P + kq
                        ps = ppool.tile([P, 1024], f32, tag="ps")
                        for db in range(2):
                            for cc in range(CC):
                                nc.tensor.matmul(
                                    ps[:, db * 512:(db + 1) * 512],
                                    a_sb[:, cc, kq * P:(kq + 1) * P],
                                    wvt[:, cc, db * 512:(db + 1) * 512],
                                    start=(cc == 0), stop=(cc == CC - 1))
                        nc.scalar.copy(
                            vproj[:, kc, :, 0:DQ],
                            ps[:].rearrange("p (h d) -> p h d", d=DQ))

            # ---- phase D: attention, one head pair at a time
            with ExitStack() as ctx:
                mpool = ctx.enter_context(tc.tile_pool(name="msk", bufs=6))
                epool = ctx.enter_context(tc.tile_pool(name="et", bufs=3))
                rpool = ctx.enter_context(tc.tile_pool(name="rz", bufs=4))
                apool = ctx.enter_context(tc.tile_pool(name="avsb", bufs=4))
                npool = ctx.enter_context(tc.tile_pool(name="nrm", bufs=2))
                avp = ctx.enter_context(
                    tc.tile_pool(name="av", bufs=1, space="PSUM"))
                zpool = ctx.enter_context(
                    tc.tile_pool(name="zb", bufs=1, space="PSUM"))

                def emit_normalize(hp_, hh_, av_sb_, rz_):
                    # PE-broadcast 1/Z (rz_ has long been ready by the time
                    # this lands in the PE queue) then normalize on DVE
                    zbb = zpool.tile([DQ, QW], f32, tag="zbb")
                    nc.tensor.matmul(zbb[:], ones[DQ:DQ + 1, 0:DQ],
                                     rz_[DQ:DQ + 1, :],
                                     start=True, stop=True,
                                     tile_position=(64, 0))
                    if hh_ == 0:
                        nc.vector.tensor_mul(attnT[0:DQ, hp_, :],
                                             zbb[:], av_sb_[0:DQ, :])
                    else:
                        nrm = npool.tile([DQ, QW], bf16, tag="nrm")
                        nc.vector.tensor_mul(nrm[:], zbb[:], av_sb_[0:DQ, :])
                        nc.sync.dma_start(attnT[DQ:P, hp_, :], nrm[:])

                pending = []
                for hp in range(HP):
                    h0, h1 = 2 * hp, 2 * hp + 1
                    av0 = avp.tile([DQ + 1, QW], f32, tag="av0")
                    av1 = avp.tile([DQ + 1, QW], f32, tag="av1")
                    # masks are applied four k-chunks per copy_predicated
                    # (amortizes the DVE per-op overhead), so AV pairs are
                    # emitted in bursts at the end of each 4-chunk group;
                    # PSUM accumulation order is irrelevant.
                    av_sched = {}
                    for kc in range(KC):
                        av_sched.setdefault(kc | 3, []).append(kc)
                    n_av = [0]

                    def emit_av(kc, ets):
                        first = n_av[0] == 0
                        n_av[0] += 1
                        last = n_av[0] == KC
                        nc.tensor.matmul(av0[:], vproj[:, kc, h0, :],
                                         ets[:, 0:QW],
                                         start=first, stop=last)
                        nc.tensor.matmul(av1[:], vproj[:, kc, h1, :],
                                         ets[:, QW:2 * QW],
                                         start=first, stop=last)

                    etq = {}
                    for kc in range(KC):
                        if kc == 3:
                            for args in pending:
                                emit_normalize(*args)
                            pending = []
                        ksl = slice(kc * P, (kc + 1) * P)
                        st = stp.tile([P, 2 * QW], f32, tag="st")
                        nc.tensor.matmul(st[:, 0:QW],
                                         kproj[0:DQ, hp, ksl],
                                         qproj[0:DQ, hp, :],
                                         start=True, stop=True,
                                         tile_position=(0, 0))
                        nc.tensor.matmul(st[:, QW:2 * QW],
                                         kproj[DQ:P, hp, ksl],
                                         qproj[DQ:P, hp, :],
                                         start=True, stop=True,
                                         tile_position=(64, 0))
                        if kc % 4 == 0:
                            msk2 = mpool.tile([P, 4, 2, QW], u8, tag="msk")
                            nc.sync.dma_start(
                                msk2[:, :, 0, :], mkt_r[:, kc // 4, h0, :, :])
                            nc.sync.dma_start(
                                msk2[:, :, 1, :], mkt_r[:, kc // 4, h1, :, :])
                        if kc % 4 == 0:
                            et4 = epool.tile([P, 4, 2 * QW], bf16, tag="et")
                        nc.scalar.activation(et4[:, kc % 4, :], st[:], Exp)
                        # masked -> exp(1e-9) = 1.0, post-exp in bf16 SBUF;
                        # one DVE op covers the whole 4-chunk group
                        if kc % 4 == 3:
                            nc.vector.copy_predicated(
                                et4[:].rearrange("p a q -> p (a q)"),
                                msk2[:].rearrange("p a h q -> p (a h q)"),
                                c1b[:, 0:1].to_broadcast([P, 8 * QW]))
                        etq[kc] = et4[:, kc % 4, :]
                        for due in sorted(av_sched.get(kc, ()), reverse=True):
                            emit_av(due, etq.pop(due))
                    for due_slot in sorted(k for k in av_sched if k >= KC):
                        for due in av_sched[due_slot]:
                            emit_av(due, etq.pop(due))
                    # drain accumulators to SBUF on ACT (frees the PSUM bank;
                    # the next pair's AV is gated by its own exp chain anyway)
                    # and kick off 1/Z on DVE; the PE-side normalize for this
                    # pair is deferred into the NEXT pair's sweep so its
                    # matmul never waits on the reciprocal.
                    ready = []
                    for hh, av in ((0, av0), (1, av1)):
                        av_sb = apool.tile([DQ + 1, QW], f32, tag="avsb")
                        nc.scalar.copy(av_sb[:], av[:])
                        rz = rpool.tile([DQ + 1, QW], f32, tag="rz")
                        # NOTE: reciprocal_approx_fast silently no-ops on HW
                        # for a base-partition-64 slice — run it on the full
                        # tile (same DVE cost; rows 0..63 are never read)
                        with nc.allow_low_precision(reason="fp32 denom"):
                            nc.vector.reciprocal_approx_fast(
                                rz[:], av_sb[:])
                        ready.append((hp, hh, av_sb, rz))
                    pending = ready
                for args in pending:
                    emit_normalize(*args)

            # ---- phase E: output projection out[q, dm] = attnT.T @ Wo
            with ExitStack() as ctx:
                opool = ctx.enter_context(tc.tile_pool(name="osb", bufs=4))
                pso = ctx.enter_context(
                    tc.tile_pool(name="pso", bufs=3, space="PSUM"))
                for qt4 in range(QW // P):
                    for db in range(2):
                        ps = pso.tile([P, 512], f32, tag="pso")
                        for hp in range(CC):
                            nc.tensor.matmul(
                                ps[:], attnT[:, hp, qt4 * P:(qt4 + 1) * P],
                                wo_sb[:, hp, db * 512:(db + 1) * 512],
                                start=(hp == 0), stop=(hp == CC - 1))
                        o_sb = opool.tile([P, 512], f32, tag="osb")
                        nc.scalar.copy(o_sb[:], ps[:])
                        nc.sync.dma_start(
                            out[qt4 * P:(qt4 + 1) * P, db * 512:(db + 1) * 512],
                            o_sb[:])
    nc.compile()
    return nc


_OFFKC = (2, 6, 10)


def prepare_in_maps(Q, K, V, mask, WQ, WK, WV, Wo):
    import ml_dtypes
    bf = ml_dtypes.bfloat16
    f8 = ml_dtypes.float8_e4m3fn
    WQ_b = np.asarray(WQ, np.float32).astype(bf)
    WK_b = np.asarray(WK, np.float32).astype(bf)
    WV_b = np.asarray(WV, np.float32).astype(bf)
    Wo_b = np.asarray(Wo, np.float32).astype(bf)
    mask_u8 = np.asarray(mask).reshape(B, L, L, H).view(np.uint8)
    kt_b = [np.ascontiguousarray(np.asarray(K, np.float32)[b].T.astype(bf))
            for b in range(B)]
    vt_b = [np.ascontiguousarray(np.asarray(V, np.float32)[b].T.astype(bf))
            for b in range(B)]
    Qf = np.asarray(Q, np.float32)
    in_maps = []
    for c in range(NC):
        b_ = c // 4
        q0 = (c % 4) * QW
        # mask[b, q, k, h] -> [h, k, q] for this core's query window
        mkt = np.ascontiguousarray(
            mask_u8[b_, q0:q0 + QW, :, :].transpose(2, 1, 0))
        in_maps.append({
            "qt": np.ascontiguousarray(Qf[b_, q0:q0 + QW, :].T.astype(bf)),
            "kt": kt_b[b_],
            "vt": vt_b[b_],
            "wq": WQ_b, "wk": WK_b, "wv": WV_b, "wo": Wo_b,
            "mkt": mkt,
        })
    return in_maps


def kernel(Q, K, V, mask, WQ, bQ, WK, bK, WV, bV, Wo, bo):
    from concourse import bass_utils

    for b_, name in ((bQ, "bQ"), (bK, "bK"), (bV, "bV"), (bo, "bo")):
        assert not np.any(np.asarray(b_)), f"{name} must be zero (setup_inputs)"

    if "nc" not in _CACHE:
        _CACHE["nc"] = _build()
    nc = _CACHE["nc"]

    in_maps = prepare_in_maps(Q, K, V, mask, WQ, WK, WV, Wo)
    res = bass_utils.run_bass_kernel_spmd(nc, in_maps, core_ids=list(range(NC)))
    out = np.empty((B, L, DM), dtype=np.float32)
    for c in range(NC):
        b_ = c // 4
        q0 = (c % 4) * QW
        out[b_, q0:q0 + QW, :] = res.results[c]["out"]
    return out


# revision 53
# speedup vs baseline: 1.1368x; 1.1368x over previous
# Multi-head attention (B=2, L=2048, D=1024, H=16, Dq=Dv=64) on 8 TRN2 NeuronCores.
#
# Sharding: data-parallel over (batch, query-rows). Core c owns batch c//4 and
# query window [(c%4)*512, (c%4)*512+512). Each core computes K/V projections
# for its batch (duplicated across the 4 cores of a batch group), its query
# projection, masked softmax attention and the output projection for its 512
# rows. No collectives; outputs are disjoint row blocks concatenated on host.
#
# v2 layout: all matmul operands in bf16 (fp32 PSUM accumulation), everything
# feature-major so the PE contracts along partitions with no on-chip
# transposes:
#   qT [d, q], kT [d, k]  (d on partitions, bf16)
#   v  [k, dv+1] bf16     (ones column makes AV row 64 accumulate Z for free)
#   S^T [k, q] = kT.T-contract-d qT  (two heads packed via PE row-tiling)
#   e = exp(S^T) bf16 (ACT, PSUM->SBUF); mask applied post-exp with one
#     copy_predicated (masked -> 1.0 == exp(1e-9), faithful to the reference)
#   AV^T [dv+1, q] accumulated over k-chunks in PSUM
#   1/Z via reciprocal_approx_fast (DVE custom op, ~18 bits), PE-broadcast
#     down to rows 0..63 via a contraction-1 f32r matmul, one tensor_mul
#   out [q, dm] = attnT.T-contract-hd Wo
# PSUM budget: st 2x[128,1024] (4 banks) + av0/av1 (2) + zbb (1) + 1 spare.
import numpy as np

B, L, DM, H, DQ = 2, 2048, 1024, 16, 64
P = 128
NC = 8
QW = (B * L) // NC          # 512 query rows per core
CC = DM // P                # 8 contraction chunks
HP = H // 2                 # 8 head pairs
KC = L // P                 # 16 key chunks

_CACHE = {}


def _build():
    import concourse.tile as tile
    from concourse import bacc, mybir

    f32 = mybir.dt.float32
    f32r = mybir.dt.float32r
    bf16 = mybir.dt.bfloat16
    u8 = mybir.dt.uint8
    fp8 = mybir.dt.float8e4
    Exp = mybir.ActivationFunctionType.Exp
    DR = mybir.MatmulPerfMode.DoubleRow

    nc = bacc.Bacc("TRN2", target_bir_lowering=False, debug=False,
                   enable_asserts=False, num_devices=NC)

    qt = nc.dram_tensor("qt", [DM, QW], bf16, kind="ExternalInput").ap()
    kt = nc.dram_tensor("kt", [DM, L], bf16, kind="ExternalInput").ap()
    vt = nc.dram_tensor("vt", [DM, L], bf16, kind="ExternalInput").ap()
    wq = nc.dram_tensor("wq", [DM, DM], bf16, kind="ExternalInput").ap()
    wk = nc.dram_tensor("wk", [DM, DM], bf16, kind="ExternalInput").ap()
    wv = nc.dram_tensor("wv", [DM, DM], bf16, kind="ExternalInput").ap()
    wo = nc.dram_tensor("wo", [DM, DM], bf16, kind="ExternalInput").ap()
    mkt = nc.dram_tensor("mkt", [H, L, QW], u8, kind="ExternalInput").ap()
    out = nc.dram_tensor("out", [QW, DM], f32, kind="ExternalOutput").ap()

    qt_r = qt.rearrange("(cc p) q -> p cc q", p=P)
    kt_r = kt.rearrange("(cc p) k -> p cc k", p=P)
    vt_r = vt.rearrange("(cc p) k -> p cc k", p=P)
    wq_r = wq.rearrange("(cc p) d -> p cc d", p=P)
    wk_r = wk.rearrange("(cc p) d -> p cc d", p=P)
    wv_r = wv.rearrange("(cc p) d -> p cc d", p=P)
    wo_r = wo.rearrange("(cc p) d -> p cc d", p=P)
    # mask grouped as [p, ko, h, kk, q]: one 3-dim DMA per head fetches 4
    # k-chunks at once (256 KB)
    mkt_r = mkt.rearrange("h (ko kk p) q -> p ko h kk q", kk=4, p=P)
    # GPSIMD mask offload abandoned: generic TensorTensor/TensorScalar BIR
    # ops fail the Pool-engine ISA check in this toolchain.
    OFFKC = ()

    with tile.TileContext(nc) as tc:
        from contextlib import ExitStack
        with ExitStack() as top:
            persist = top.enter_context(tc.tile_pool(name="persist", bufs=1))
            # v projection with an extra all-ones column: AV matmul row 64
            # then accumulates Z = sum_k e[k, q] for free.
            vproj = persist.tile([P, KC, H, DQ + 1], bf16)   # 32.5 KB/part
            kproj = persist.tile([P, CC, L], bf16)           # 32 KB/part
            qproj = persist.tile([P, CC, QW], bf16)          # 8 KB/part
            attnT = persist.tile([P, HP, QW], bf16)          # 8 KB/part
            wo_sb = persist.tile([P, CC, DM], bf16)          # 16 KB/part
            ones = persist.tile([P, DQ], f32)
            c1f = persist.tile([P, 1], f32)
            c1b = persist.tile([P, 1], bf16)
            nc.vector.memset(ones[:], 1.0)
            nc.vector.memset(c1f[:], 1.0)
            nc.vector.memset(c1b[:], 1.0)
            nc.vector.tensor_copy(
                vproj[:, :, :, DQ:DQ + 1],
                c1f[:, 0:1].to_broadcast([P, KC, H, 1]))
            for cc in range(CC):
                nc.sync.dma_start(wo_sb[:, cc, :], wo_r[:, cc, :])

            # ---- phases A/B/C: projections, weight-stationary, long streams
            with ExitStack() as ctx:
                wpool = ctx.enter_context(tc.tile_pool(name="wstage", bufs=2))
                spool = ctx.enter_context(tc.tile_pool(name="astage", bufs=2))
                ppool = ctx.enter_context(
                    tc.tile_pool(name="pproj", bufs=3, space="PSUM"))

                # phase A: qproj[d, q] = WQ.T-contract-c Q^T
                wqt = wpool.tile([P, CC, DM], bf16, tag="w", name="wq")
                for cc in range(CC):
                    nc.sync.dma_start(wqt[:, cc, :], wq_r[:, cc, :])
                a0 = spool.tile([P, CC, 2 * QW], bf16, tag="act", name="aq")
                for cc in range(CC):
                    nc.sync.dma_start(a0[:, cc, 0:QW], qt_r[:, cc, :])
                for dp in range(CC // 2):
                    ps = ppool.tile([P, 1024], f32, tag="ps")
                    for half in range(2):
                        dc = 2 * dp + half
                        sl = slice(half * QW, (half + 1) * QW)
                        for cc in range(CC):
                            nc.tensor.matmul(ps[:, sl],
                                             wqt[:, cc, dc * P:(dc + 1) * P],
                                             a0[:, cc, 0:QW],
                                             start=(cc == 0), stop=(cc == CC - 1))
                    nc.scalar.copy(
                        qproj[:, 2 * dp:2 * dp + 2, :],
                        ps[:].rearrange("p (c q) -> p c q", c=2))

                # phase B: kproj[d, k], k-blocks of 1024 (bf16 streams)
                wkt = wpool.tile([P, CC, DM], bf16, tag="w", name="wk")
                for cc in range(CC):
                    nc.sync.dma_start(wkt[:, cc, :], wk_r[:, cc, :])
                KB = 1024
                for kb in range(L // KB):
                    ksl = slice(kb * KB, (kb + 1) * KB)
                    a_sb = spool.tile([P, CC, KB], bf16, tag="act")
                    nc.sync.dma_start(a_sb[:], kt_r[:, :, ksl])
                    for dc in range(CC):
                        ps = ppool.tile([P, KB], f32, tag="ps")
                        for half in range(2):
                            hs = slice(half * 512, (half + 1) * 512)
                            for cc in range(CC):
                                nc.tensor.matmul(ps[:, hs],
                                                 wkt[:, cc, dc * P:(dc + 1) * P],
                                                 a_sb[:, cc, hs],
                                                 start=(cc == 0), stop=(cc == CC - 1))
                        nc.scalar.copy(kproj[:, dc, ksl], ps[:])

                # phase C: v[k, dv] = V^T as lhsT, WV as rhs
                wvt = wpool.tile([P, CC, DM], bf16, tag="w", name="wv")
                for cc in range(CC):
                    nc.sync.dma_start(wvt[:, cc, :], wv_r[:, cc, :])
                for kb in range(L // KB):
                    a_sb = spool.tile([P, CC, KB], bf16, tag="act")
                    nc.sync.dma_start(a_sb[:], vt_r[:, :, kb * KB:(kb + 1) * KB])
                    for kq in range(KB // P):
                        kc = (kb * KB) // P + kq
                        ps = ppool.tile([P, 1024], f32, tag="ps")
                        for db in range(2):
                            for cc in range(CC):
                                nc.tensor.matmul(
                                    ps[:, db * 512:(db + 1) * 512],
                                    a_sb[:, cc, kq * P:(kq + 1) * P],
                                    wvt[:, cc, db * 512:(db + 1) * 512],
                                    start=(cc == 0), stop=(cc == CC - 1))
                        nc.scalar.copy(
                            vproj[:, kc, :, 0:DQ],
                            ps[:].rearrange("p (h d) -> p h d", d=DQ))

            # ---- phase D: attention, one head pair at a time
            with ExitStack() as ctx:
                mpool = ctx.enter_context(tc.tile_pool(name="msk", bufs=6))
                epool = ctx.enter_context(tc.tile_pool(name="et", bufs=6))
                rpool = ctx.enter_context(tc.tile_pool(name="rz", bufs=4))
                apool = ctx.enter_context(tc.tile_pool(name="avsb", bufs=4))
                npool = ctx.enter_context(tc.tile_pool(name="nrm", bufs=2))
                stp = ctx.enter_context(
                    tc.tile_pool(name="st", bufs=2, space="PSUM"))
                avp = ctx.enter_context(
                    tc.tile_pool(name="av", bufs=1, space="PSUM"))
                zpool = ctx.enter_context(
                    tc.tile_pool(name="zb", bufs=1, space="PSUM"))

                def emit_normalize(hp_, hh_, av_sb_, rz_):
                    # PE-broadcast 1/Z (rz_ has long been ready by the time
                    # this lands in the PE queue) then normalize on DVE
                    zbb = zpool.tile([DQ, QW], f32, tag="zbb")
                    nc.tensor.matmul(zbb[:], ones[DQ:DQ + 1, 0:DQ],
                                     rz_[DQ:DQ + 1, :],
                                     start=True, stop=True,
                                     tile_position=(64, 0))
                    if hh_ == 0:
                        nc.vector.tensor_mul(attnT[0:DQ, hp_, :],
                                             zbb[:], av_sb_[0:DQ, :])
                    else:
                        nrm = npool.tile([DQ, QW], bf16, tag="nrm")
                        nc.vector.tensor_mul(nrm[:], zbb[:], av_sb_[0:DQ, :])
                        nc.sync.dma_start(attnT[DQ:P, hp_, :], nrm[:])

                pending = []
                for hp in range(HP):
                    h0, h1 = 2 * hp, 2 * hp + 1
                    av0 = avp.tile([DQ + 1, QW], f32, tag="av0")
                    av1 = avp.tile([DQ + 1, QW], f32, tag="av1")
                    # masks are applied two k-chunks per copy_predicated
                    # (halves the DVE per-op overhead), so each even chunk's
                    # AV pair is emitted one iteration late; PSUM
                    # accumulation order is irrelevant.
                    av_sched = {}
                    for kc in range(KC):
                        av_sched.setdefault(
                            kc + 1 if kc % 2 == 0 else kc, []).append(kc)
                    n_av = [0]

                    def emit_av(kc, ets):
                        first = n_av[0] == 0
                        n_av[0] += 1
                        last = n_av[0] == KC
                        nc.tensor.matmul(av0[:], vproj[:, kc, h0, :],
                                         ets[:, 0:QW],
                                         start=first, stop=last)
                        nc.tensor.matmul(av1[:], vproj[:, kc, h1, :],
                                         ets[:, QW:2 * QW],
                                         start=first, stop=last)

                    etq = {}
                    for kc in range(KC):
                        # flush the previous pair's deferred normalize early:
                        # its DVE muls then run inside the pair-boundary
                        # bubble (DVE otherwise idles waiting for this pair's
                        # first scores->exp chain)
                        if kc == 1:
                            for args in pending:
                                emit_normalize(*args)
                            pending = []
                        ksl = slice(kc * P, (kc + 1) * P)
                        st = stp.tile([P, 2 * QW], f32, tag="st")
                        nc.tensor.matmul(st[:, 0:QW],
                                         kproj[0:DQ, hp, ksl],
                                         qproj[0:DQ, hp, :],
                                         start=True, stop=True,
                                         tile_position=(0, 0))
                        nc.tensor.matmul(st[:, QW:2 * QW],
                                         kproj[DQ:P, hp, ksl],
                                         qproj[DQ:P, hp, :],
                                         start=True, stop=True,
                                         tile_position=(64, 0))
                        if kc % 4 == 0:
                            msk2 = mpool.tile([P, 4, 2, QW], u8, tag="msk")
                            nc.sync.dma_start(
                                msk2[:, :, 0, :], mkt_r[:, kc // 4, h0, :, :])
                            nc.sync.dma_start(
                                msk2[:, :, 1, :], mkt_r[:, kc // 4, h1, :, :])
                        if kc % 2 == 0:
                            et2 = epool.tile([P, 2, 2 * QW], bf16, tag="et")
                        nc.scalar.activation(et2[:, kc % 2, :], st[:], Exp)
                        # masked -> exp(1e-9) = 1.0, post-exp in bf16 SBUF;
                        # one DVE op covers both chunks of the even/odd pair
                        if kc % 2 == 1:
                            e0 = kc % 4 - 1
                            nc.vector.copy_predicated(
                                et2[:].rearrange("p a q -> p (a q)"),
                                msk2[:, e0:e0 + 2, :, :].rearrange(
                                    "p a h q -> p (a h q)"),
                                c1b[:, 0:1].to_broadcast([P, 4 * QW]))
                        etq[kc] = et2[:, kc % 2, :]
                        for due in sorted(av_sched.get(kc, ()), reverse=True):
                            emit_av(due, etq.pop(due))
                    for due_slot in sorted(k for k in av_sched if k >= KC):
                        for due in av_sched[due_slot]:
                            emit_av(due, etq.pop(due))
                    # drain accumulators to SBUF on ACT (frees the PSUM bank;
                    # the next pair's AV is gated by its own exp chain anyway)
                    # and kick off 1/Z on DVE; the PE-side normalize for this
                    # pair is deferred into the NEXT pair's sweep so its
                    # matmul never waits on the reciprocal.
                    ready = []
                    for hh, av in ((0, av0), (1, av1)):
                        av_sb = apool.tile([DQ + 1, QW], f32, tag="avsb")
                        nc.scalar.copy(av_sb[:], av[:])
                        rz = rpool.tile([DQ + 1, QW], f32, tag="rz")
                        # NOTE: reciprocal_approx_fast silently no-ops on HW
                        # for a base-partition-64 slice — run it on the full
                        # tile (same DVE cost; rows 0..63 are never read)
                        with nc.allow_low_precision(reason="fp32 denom"):
                            nc.vector.reciprocal_approx_fast(
                                rz[:], av_sb[:])
                        ready.append((hp, hh, av_sb, rz))
                    pending = ready
                for args in pending:
                    emit_normalize(*args)

            # ---- phase E: output projection out[q, dm] = attnT.T @ Wo
            with ExitStack() as ctx:
                opool = ctx.enter_context(tc.tile_pool(name="osb", bufs=4))
                pso = ctx.enter_context(
                    tc.tile_pool(name="pso", bufs=3, space="PSUM"))
                for qt4 in range(QW // P):
                    for db in range(2):
                        ps = pso.tile([P, 512], f32, tag="pso")
                        for hp in range(CC):
                            nc.tensor.matmul(
                                ps[:], attnT[:, hp, qt4 * P:(qt4 + 1) * P],
                                wo_sb[:, hp, db * 512:(db + 1) * 512],
                                start=(hp == 0), stop=(hp == CC - 1))
                        o_sb = opool.tile([P, 512], f32, tag="osb")
                        nc.scalar.copy(o_sb[:], ps[:])
                        nc.sync.dma_start(
                            out[qt4 * P:(qt4 + 1) * P, db * 512:(db + 1) * 512],
                            o_sb[:])
    nc.compile()
    return nc


_OFFKC = (2, 6, 10)


def prepare_in_maps(Q, K, V, mask, WQ, WK, WV, Wo):
    import ml_dtypes
    bf = ml_dtypes.bfloat16
    f8 = ml_dtypes.float8_e4m3fn
    WQ_b = np.asarray(WQ, np.float32).astype(bf)
    WK_b = np.asarray(WK, np.float32).astype(bf)
    WV_b = np.asarray(WV, np.float32).astype(bf)
    Wo_b = np.asarray(Wo, np.float32).astype(bf)
    mask_u8 = np.asarray(mask).reshape(B, L, L, H).view(np.uint8)
    kt_b = [np.ascontiguousarray(np.asarray(K, np.float32)[b].T.astype(bf))
            for b in range(B)]
    vt_b = [np.ascontiguousarray(np.asarray(V, np.float32)[b].T.astype(bf))
            for b in range(B)]
    Qf = np.asarray(Q, np.float32)
    in_maps = []
    for c in range(NC):
        b_ = c // 4
        q0 = (c % 4) * QW
        # mask[b, q, k, h] -> [h, k, q] for this core's query window
        mkt = np.ascontiguousarray(
            mask_u8[b_, q0:q0 + QW, :, :].transpose(2, 1, 0))
        in_maps.append({
            "qt": np.ascontiguousarray(Qf[b_, q0:q0 + QW, :].T.astype(bf)),
            "kt": kt_b[b_],
            "vt": vt_b[b_],
            "wq": WQ_b, "wk": WK_b, "wv": WV_b, "wo": Wo_b,
            "mkt": mkt,
        })
    return in_maps


def kernel(Q, K, V, mask, WQ, bQ, WK, bK, WV, bV, Wo, bo):
    from concourse import bass_utils

    for b_, name in ((bQ, "bQ"), (bK, "bK"), (bV, "bV"), (bo, "bo")):
        assert not np.any(np.asarray(b_)), f"{name} must be zero (setup_inputs)"

    if "nc" not in _CACHE:
        _CACHE["nc"] = _build()
    nc = _CACHE["nc"]

    in_maps = prepare_in_maps(Q, K, V, mask, WQ, WK, WV, Wo)
    res = bass_utils.run_bass_kernel_spmd(nc, in_maps, core_ids=list(range(NC)))
    out = np.empty((B, L, DM), dtype=np.float32)
    for c in range(NC):
        b_ = c // 4
        q0 = (c % 4) * QW
        out[b_, q0:q0 + QW, :] = res.results[c]["out"]
    return out
